# revision 23
# baseline (speedup 1.0000x reference)
"""Multi-head causal attention (B=2, S=2048, D=1024, H=16) on 8 trn2 NeuronCores.

Sharding: core c handles batch b = c//4 and head group g = c%4 (heads 4g..4g+3).
Each core computes qkv projection, causal attention (scoresT layout) and the
partial output projection for its 4 heads; the host sums the 4 partials per
batch.

Schedule (v4), designed around measured engine costs (matmul ~N/2.4GHz
back-to-back, ACTIVATE ~(N+352)/1.2ns, DVE ~250ns/op overhead, HAM power
throttle under sustained PE activity):
  - Scalar engine runs ONLY exp: one strided [128, 2, 512-lo] ACTIVATE per
    (head-pair, key-tile) out of a 2-bank PSUM scores group holding both
    heads of the pair side by side.
  - Scores matmuls of a head pair go to PE row groups 0-63/64-127 and are
    emitted back-to-back so they run CONCURRENTLY on the split array (K=64).
  - Input DMAs are split across both HWDGE queues (SP=weights, ACT=x).
  - ctx accumulates per head in [65,512] PSUM tiles; softmax denominators are
    copied to SBUF, reciprocal'd with reciprocal_approx_fast, broadcast once
    per pair on gpsimd ([64,1024]), applied on DVE.
  - qkv projection of block nb+1 / out-projection of block qb-1 interleave
    into attention(qb) chunks (one fill per chunk, adaptively more when
    backlogged); the final out-projection double-buffers through the (by
    then free) scores PSUM pool with evacuations split scalar/vector.
  - 1/sqrt(HD) is folded into wq on the host.
"""

import sys
from contextlib import ExitStack

for _p in ("/opt/trn_rl_repo",):
    if _p not in sys.path:
        sys.path.insert(0, _p)

import numpy as np

import concourse.bass as bass  # noqa: F401
import concourse.tile as tile
from concourse import bacc, bass_utils, mybir

B, S, D, H, HD = 2, 2048, 1024, 16, 64
P = 128
NCORES = 8
NT = S // P          # 16 token tiles
KD = D // P          # 8 contraction tiles over D
NB = S // 512        # 4 query blocks of 512
HPC = 4              # heads per core
WCOLS = HPC * HD     # 256 weight columns per core per q/k/v

F32 = mybir.dt.float32
BF16 = mybir.dt.bfloat16
EXP = mybir.ActivationFunctionType.Exp

DT = BF16


def prep(x: np.ndarray) -> np.ndarray:
    import ml_dtypes

    return np.ascontiguousarray(x, np.float32).astype(ml_dtypes.bfloat16)


def _emit(tc: tile.TileContext, aps: dict):
    nc = tc.nc
    xT, wq, wk, wv, wo, tri, out = (
        aps["xT"], aps["wq"], aps["wk"], aps["wv"], aps["wo"],
        aps["tri"], aps["out"],
    )

    with ExitStack() as top:
        qk_pool = top.enter_context(tc.tile_pool(name="qk", bufs=4))
        v_pool = top.enter_context(tc.tile_pool(name="v1", bufs=NT))
        ctx_pool = top.enter_context(tc.tile_pool(name="ctxT", bufs=2))
        wo_pool = top.enter_context(tc.tile_pool(name="wo", bufs=2))
        const_pool = top.enter_context(tc.tile_pool(name="const", bufs=1))
        small_pool = top.enter_context(tc.tile_pool(name="small", bufs=4))
        out_pool = top.enter_context(tc.tile_pool(name="outsb", bufs=3))
        exp_pool = top.enter_context(tc.tile_pool(name="expT", bufs=8))
        x_pool = top.enter_context(tc.tile_pool(name="xc", bufs=NB))
        w_pool = top.enter_context(tc.tile_pool(name="w", bufs=3))
        # PSUM: sc 2x[128,1024] (banks 0-3), ctx 2x[65,512] (banks 4-5),
        # pp 2x[128,512] shared by qkv-proj + out-proj fills (banks 6-7)
        sc_pool = top.enter_context(tc.tile_pool(name="sc", bufs=2, space="PSUM"))
        ctxps_pool = top.enter_context(
            tc.tile_pool(name="ctxps", bufs=2, space="PSUM")
        )
        pp_pool = top.enter_context(tc.tile_pool(name="pp", bufs=2, space="PSUM"))

        # persistent SBUF tiles
        qT = [qk_pool.tile([P, S], DT, tag="qk", name=f"qT{i}") for i in range(2)]
        kT = [qk_pool.tile([P, S], DT, tag="qk", name=f"kT{i}") for i in range(2)]
        v1 = [
            v_pool.tile([P, HPC * (HD + 1)], DT, tag="v1", name=f"v1_{i}")
            for i in range(NT)
        ]
        ctxT = [
            ctx_pool.tile([P, S], DT, tag="ctxT", name=f"ctxT{i}")
            for i in range(2)
        ]
        wo_sb = [wo_pool.tile([P, D], DT, tag="wo", name=f"wo{i}") for i in range(2)]
        tri_sb = const_pool.tile([P, P], DT, tag="tri")

        w_all = {
            n: w_pool.tile([P, KD * WCOLS], DT, tag="w", name=f"w_{n}")
            for n in ("q", "k", "v")
        }
        wq_sb = [w_all["q"][:, kt * WCOLS : (kt + 1) * WCOLS] for kt in range(KD)]
        wk_sb = [w_all["k"][:, kt * WCOLS : (kt + 1) * WCOLS] for kt in range(KD)]
        wv_sb = [w_all["v"][:, kt * WCOLS : (kt + 1) * WCOLS] for kt in range(KD)]
        xc_all = [
            x_pool.tile([P, KD * 512], DT, tag="xc", name=f"xca{nb}")
            for nb in range(NB)
        ]
        xc = {
            (kt, nb): xc_all[nb][:, kt * 512 : (kt + 1) * 512]
            for kt in range(KD)
            for nb in range(NB)
        }

        # DMA priority order: the first fill (q-proj of block 0) needs only
        # w_q + xc block 0, so those lead on the two HWDGE queues (xc0 split
        # across both); everything else queues behind. The prologue critical
        # path is HBM-bound, so ordering — not queue count — sets when the
        # PE can start.
        nc.sync.dma_start(w_all["q"][:], wq[:])
        half = KD * 512 // 2
        nc.scalar.dma_start(xc_all[0][:, 0:half], xT[:, 0:half])
        nc.sync.dma_start(xc_all[0][:, half : KD * 512], xT[:, half : KD * 512])
        nc.sync.dma_start(w_all["k"][:], wk[:])
        nc.sync.dma_start(tri_sb[:], tri[:])
        nc.sync.dma_start(w_all["v"][:], wv[:])
        for nb in range(1, NB):
            nc.scalar.dma_start(
                xc_all[nb][:], xT[:, nb * KD * 512 : (nb + 1) * KD * 512]
            )
        for i in range(2):
            nc.sync.dma_start(wo_sb[i][:], wo[i * P : (i + 1) * P, :])

        # ones column of v1: memset whole tile once, value region is
        # overwritten by the v-projection evacuations afterwards
        for tt in range(NT):
            nc.vector.memset(v1[tt][:], 1.0)

        # ---- qkv projection fills, split into ~850ns pieces ----------------
        # Each fill is two adjacent pieces (4 contraction tiles each) holding
        # one pp PSUM tile across the pair; the scheduler keeps pieces of a
        # fill consecutive so no other pp allocation lands in between.
        def qk_fill(w_sb, dstT, p, nb):
            st = {}

            def go1():
                st["ps"] = pp_pool.tile([P, 512], F32, tag="pp", name="pp_ps")
                for kt in range(KD // 2):
                    nc.tensor.matmul(
                        st["ps"][:],
                        w_sb[kt][:, p * P : (p + 1) * P],
                        xc[(kt, nb)][:],
                        start=(kt == 0),
                        stop=False,
                    )

            def go2():
                ps = st["ps"]
                for kt in range(KD // 2, KD):
                    nc.tensor.matmul(
                        ps[:],
                        w_sb[kt][:, p * P : (p + 1) * P],
                        xc[(kt, nb)][:],
                        start=False,
                        stop=(kt == KD - 1),
                    )
                nc.vector.tensor_copy(
                    dstT[p][:, nb * 512 : (nb + 1) * 512], ps[:]
                )

            return [go1, go2]

        def v_fill(nb, tloc):
            st = {}

            def go1():
                st["ps"] = pp_pool.tile([P, 512], F32, tag="pp", name="pp_ps")
                for kt in range(KD // 2):
                    nc.tensor.matmul(
                        st["ps"][:, 0:WCOLS],
                        xc[(kt, nb)][:, tloc * P : (tloc + 1) * P],
                        wv_sb[kt][:],
                        start=(kt == 0),
                        stop=False,
                    )

            def go2():
                ps = st["ps"]
                tt = nb * 4 + tloc
                for kt in range(KD // 2, KD):
                    nc.tensor.matmul(
                        ps[:, 0:WCOLS],
                        xc[(kt, nb)][:, tloc * P : (tloc + 1) * P],
                        wv_sb[kt][:],
                        start=False,
                        stop=(kt == KD - 1),
                    )
                v1_view = v1[tt][:].rearrange("p (a c) -> p a c", c=HD + 1)
                nc.vector.tensor_copy(
                    v1_view[:, :, 0:HD],
                    ps[:, 0:WCOLS].rearrange("p (a c) -> p a c", c=HD),
                )

            return [go1, go2]

        # ---- out-projection fills for query block qb -----------------------
        def outproj_fills(qb, final=False, evac="vector"):
            fills = []

            def o_fill(tt, i):
                def go():
                    if final:
                        pso = sc_pool.tile([P, 1024], F32, tag="sc")
                        osb = out_pool.tile([P, 1024], DT, tag="osb")
                        # per-half: 2 mms, then evac (scalar/vector alternate)
                        # and DMA immediately so the tail pipelines
                        for ob in range(2):
                            for kt2 in range(2):
                                nc.tensor.matmul(
                                    pso[:, ob * 512 : ob * 512 + 512],
                                    ctxT[kt2][:, tt * P : (tt + 1) * P],
                                    wo_sb[kt2][:, ob * 512 : (ob + 1) * 512],
                                    start=(kt2 == 0),
                                    stop=(kt2 == 1),
                                )
                            half = slice(ob * 512, ob * 512 + 512)
                            if ob == 0:
                                nc.scalar.copy(osb[:, half], pso[:, half])
                                nc.scalar.dma_start(
                                    out[tt * P : (tt + 1) * P, half], osb[:, half]
                                )
                            else:
                                nc.vector.tensor_copy(osb[:, half], pso[:, half])
                                nc.sync.dma_start(
                                    out[tt * P : (tt + 1) * P, half], osb[:, half]
                                )
                    else:
                        ob = i % 2
                        pso = pp_pool.tile([P, 512], F32, tag="pp")
                        for kt2 in range(2):
                            nc.tensor.matmul(
                                pso[:],
                                ctxT[kt2][:, tt * P : (tt + 1) * P],
                                wo_sb[kt2][:, ob * 512 : (ob + 1) * 512],
                                start=(kt2 == 0),
                                stop=(kt2 == 1),
                            )
                        osb = out_pool.tile([P, 512], DT, tag="osbh")
                        if evac == "scalar":
                            # tail fills: ACT is idle and the DVE is busy
                            # with the final normalize chain — evacuating on
                            # scalar keeps the pp PSUM pair recycling
                            nc.scalar.copy(osb[:], pso[:])
                            nc.scalar.dma_start(
                                out[tt * P : (tt + 1) * P, ob * 512 : (ob + 1) * 512],
                                osb[:],
                            )
                        else:
                            nc.vector.tensor_copy(osb[:], pso[:])
                            nc.sync.dma_start(
                                out[tt * P : (tt + 1) * P, ob * 512 : (ob + 1) * 512],
                                osb[:],
                            )

                return go

            if final:
                fills.append(o_fill(qb * 4 + 3, 0))
            else:
                # one closure per 512-half: finer interleave into chunks
                for i in range(8):
                    fills.append(o_fill(qb * 4 + i // 2, i))
            return fills

        # ---- attention chunk pieces (software-pipelined stream) ------------
        # Per chunk (2 key tiles): scores into 2x[128,1024] PSUM (per-jt, so
        # the ACT exps chain back-to-back), one [128,2048] et tile shared by
        # both jts (enables a single batched tri-mask per diag chunk).
        def emit_scores(desc):
            qb, pair, jt0 = desc
            q0 = qb * 512
            scps = []
            for jt in (jt0, jt0 + 1):
                m = jt - 4 * qb
                lo = P * m if m > 0 else 0
                scp = sc_pool.tile([P, 1024], F32, tag="sc")
                for off in (0, 64):
                    nc.tensor.matmul(
                        scp[:, (off // 64) * 512 + lo : (off // 64) * 512 + 512],
                        kT[pair][off : off + 64, jt * P : (jt + 1) * P],
                        qT[pair][off : off + 64, q0 + lo : q0 + 512],
                        start=True,
                        stop=True,
                    )
                scps.append((scp, lo))
            return scps

        def emit_exp(desc, scps):
            qb, pair, jt0 = desc
            et = exp_pool.tile([P, 2048], DT, tag="expT")
            for idx, (scp, lo) in enumerate(scps):
                scv = scp[:].rearrange("p (h n) -> p h n", h=2)
                etv = et[:, idx * 1024 : (idx + 1) * 1024].rearrange(
                    "p (h n) -> p h n", h=2
                )
                nc.scalar.activation(etv[:, :, lo:512], scv[:, :, lo:512], EXP)
            if jt0 - 4 * qb >= 0:
                # diag chunk: one batched triangle multiply over
                # [128, jt, head, 128]; jt stride 1024+128 lands each jt at
                # its own lo (lo1 = lo0+128)
                lo0 = scps[0][1]
                ev = bass.AP(
                    et[:].tensor, et[:].offset + lo0,
                    [[2048, P], [1152, 2], [512, 2], [1, P]],
                )
                tb = bass.AP(
                    tri_sb[:].tensor, tri_sb[:].offset,
                    [[P, P], [0, 2], [0, 2], [1, P]],
                )
                nc.vector.tensor_mul(ev, ev, tb)
            return et

        def emit_attnv(desc, et, ctxAB):
            qb, pair, jt0 = desc
            njt = 4 * qb + 4
            ctxA, ctxB = ctxAB
            for idx, jt in enumerate((jt0, jt0 + 1)):
                m = jt - 4 * qb
                lo = P * m if m > 0 else 0
                for half, ctx in ((0, ctxA), (1, ctxB)):
                    h = 2 * pair + half
                    nc.tensor.matmul(
                        ctx[:, lo:512],
                        v1[jt][:, h * 65 : (h + 1) * 65],
                        et[:, idx * 1024 + half * 512 + lo : idx * 1024 + half * 512 + 512],
                        start=(jt == 0),
                        stop=(jt == njt - 1),
                        skip_group_check=True,
                    )

        # normalize, split in two pieces: the reciprocal+broadcast issue right
        # after the pair's last attn@v, the multiplies one chunk later so
        # their gpsimd-wait never blocks the DVE FIFO (which would stall the
        # exp chain through et-tile recycling)
        def emit_normalize1(qb, pair, ctxAB):
            ctxA, ctxB = ctxAB
            rec = small_pool.tile([1, 1024], F32, tag="rec")
            nc.vector.tensor_copy(rec[:, 0:512], ctxA[64:65, :])
            nc.vector.tensor_copy(rec[:, 512:1024], ctxB[64:65, :])
            reci = small_pool.tile([1, 1024], F32, tag="reci")
            nc.vector.reciprocal_approx_fast(reci[:], rec[:])
            recb = small_pool.tile([64, 1024], F32, tag="recb")
            nc.gpsimd.partition_broadcast(recb[:], reci[:], channels=64)
            return recb

        def emit_normalize2(qb, pair, ctxAB, recb):
            q0 = qb * 512
            ctxA, ctxB = ctxAB
            for half, ctx in ((0, ctxA), (1, ctxB)):
                nc.vector.tensor_mul(
                    ctxT[pair][64 * half : 64 * half + 64, q0 : q0 + 512],
                    ctx[0:64, :],
                    recb[:, half * 512 : half * 512 + 512],
                )

        # ===== schedule ======================================================
        # One flattened chunk stream across all (qb, pair); scores+exp of
        # chunk i+1 are emitted BEFORE attn@v of chunk i, so the PE FIFO
        # keeps the next chunk's scores ahead of the current consumption and
        # the ACT exp chain never waits on attn@v. Fill pieces carry
        # (release, deadline): release keeps them from FIFO-blocking the PE
        # on not-yet-landed DMAs / not-yet-emitted normalizes, deadline
        # force-emits them before the chunk that consumes their output.
        # outproj(2) is reserved to cover the final normalize chain.
        descs = [
            (qb, pair, jt0)
            for qb in range(NB)
            for pair in range(2)
            for jt0 in range(0, 4 * qb + 4, 2)
        ]
        DIX = {d: ci for ci, d in enumerate(descs)}
        nlast = {}  # (qb, pair) -> last chunk index in stream
        qb_first = {}
        for ci, (qb, pair, jt0) in enumerate(descs):
            nlast[(qb, pair)] = ci
            qb_first.setdefault(qb, ci)

        INF = 10**9
        sched = []  # [release, deadline, fn] in emission order

        def add(rel, dl, fns):
            for fn in fns:
                sched.append((rel, dl, fn))

        # projection fills: release when the previous block's attention
        # starts (their x DMA is long done by then), deadline just before
        # their first reader in the stream
        for nb in range(NB):
            rel = 0 if nb <= 1 else qb_first[nb - 1]
            if nb > 0:
                add(rel, DIX[(nb, 0, 0)] - 1, qk_fill(wq_sb, qT, 0, nb))
                add(rel, DIX[(nb, 0, 0)] - 1, qk_fill(wk_sb, kT, 0, nb))
            for tloc in range(4):
                jt = 4 * nb + tloc
                add(rel, DIX[(nb, 0, jt - jt % 2)], v_fill(nb, tloc))
            add(rel, DIX[(nb, 1, 0)] - 1, qk_fill(wq_sb, qT, 1, nb))
            add(rel, DIX[(nb, 1, 0)] - 1, qk_fill(wk_sb, kT, 1, nb))
        # out-projections of finished blocks: released two chunks after the
        # block's pair-1 normalize is emitted, no deadline
        for qb2 in range(2):
            add(nlast[(qb2, 1)] + 2, INF, outproj_fills(qb2))
        tail_fills = outproj_fills(2, evac="scalar")

        # prologue: q/k projection of block 0 pair 0 only, then the first
        # chunk's scores+exp can issue
        for fn in qk_fill(wq_sb, qT, 0, 0) + qk_fill(wk_sb, kT, 0, 0):
            fn()

        ctx_tiles = {}

        def ctx_for(qb, pair):
            if (qb, pair) not in ctx_tiles:
                ctxA = ctxps_pool.tile([65, 512], F32, tag="ctxps")
                ctxB = ctxps_pool.tile([65, 512], F32, tag="ctxps")
                ctx_tiles[(qb, pair)] = (ctxA, ctxB)
            return ctx_tiles[(qb, pair)]

        pending_norm = []
        ets = {}
        # prologue: scores+exp of chunk 0
        ets[0] = emit_exp(descs[0], emit_scores(descs[0]))
        for ci, desc in enumerate(descs):
            # fill pieces: forced by deadline, paced by earliest deadline /
            # even spread; popped strictly from the front so the two pieces
            # of one fill never get another pp allocation between them
            take = 0
            navail = 0
            for rel, dl, fn in sched:
                if rel > ci:
                    break
                navail += 1
            for j in range(navail):
                if sched[j][1] <= ci:
                    take = j + 1
            pace = 0
            for j in range(navail):
                dl = sched[j][1]
                if dl < INF:
                    pace = -(-(j + 1) // max(1, dl - ci + 1))
                    break
            total_left = max(1, len(descs) - ci)
            pace = max(pace, -(-navail // total_left))
            take = min(navail, max(take, pace))
            for _ in range(take):
                sched.pop(0)[2]()
            # scores + exp for the next chunk, ahead of this chunk's attn@v
            if ci + 1 < len(descs):
                ets[ci + 1] = emit_exp(descs[ci + 1], emit_scores(descs[ci + 1]))
            # deferred normalize multiplies (ahead of the next pair's first
            # attn@v, whose ctx-tile allocation reuses the PSUM slots)
            while pending_norm:
                emit_normalize2(*pending_norm.pop(0))
            # attn@v for this chunk
            emit_attnv(desc, ets.pop(ci), ctx_for(desc[0], desc[1]))
            if ci == nlast[(desc[0], desc[1])]:
                key = (desc[0], desc[1], ctx_for(desc[0], desc[1]))
                recb = emit_normalize1(*key)
                pending_norm.append((*key, recb))
        for item in sched:  # anything not consumed by the chunk slots
            item[2]()
        while pending_norm:
            emit_normalize2(*pending_norm.pop(0))
        for f in tail_fills:  # reserved: runs during the last normalize chain
            f()
        # final out-projection. The kt2=0 half only needs pair-0's ctx
        # (ready before pair-1's normalize chain finishes), so pre-issue it
        # into held PSUM slots to keep the PE busy during that chain; then
        # finish kt2=1 + evac + DMA per token tile.
        held = []
        for tt in (12, 13):
            pso = sc_pool.tile([P, 1024], F32, tag="sc")
            for ob in range(2):
                nc.tensor.matmul(
                    pso[:, ob * 512 : ob * 512 + 512],
                    ctxT[0][:, tt * P : (tt + 1) * P],
                    wo_sb[0][:, ob * 512 : (ob + 1) * 512],
                    start=True,
                    stop=False,
                    skip_group_check=True,
                )
            held.append((tt, pso, 1024))
        for ob in range(2):
            pp = pp_pool.tile([P, 512], F32, tag="pp")
            nc.tensor.matmul(
                pp[:],
                ctxT[0][:, 14 * P : 15 * P],
                wo_sb[0][:, ob * 512 : (ob + 1) * 512],
                start=True,
                stop=False,
                skip_group_check=True,
            )
            held.append((14, pp, ob))
        for tt, pso, kind in held:
            if kind == 1024:
                osb = out_pool.tile([P, 1024], DT, tag="osb")
                for ob in range(2):
                    nc.tensor.matmul(
                        pso[:, ob * 512 : ob * 512 + 512],
                        ctxT[1][:, tt * P : (tt + 1) * P],
                        wo_sb[1][:, ob * 512 : (ob + 1) * 512],
                        start=False,
                        stop=True,
                        skip_group_check=True,
                    )
                    half = slice(ob * 512, ob * 512 + 512)
                    if ob == 0:
                        nc.scalar.copy(osb[:, half], pso[:, half])
                        nc.scalar.dma_start(
                            out[tt * P : (tt + 1) * P, half], osb[:, half]
                        )
                    else:
                        nc.vector.tensor_copy(osb[:, half], pso[:, half])
                        nc.sync.dma_start(
                            out[tt * P : (tt + 1) * P, half], osb[:, half]
                        )
            else:
                ob = kind
                half = slice(ob * 512, ob * 512 + 512)
                nc.tensor.matmul(
                    pso[:],
                    ctxT[1][:, tt * P : (tt + 1) * P],
                    wo_sb[1][:, ob * 512 : (ob + 1) * 512],
                    start=False,
                    stop=True,
                    skip_group_check=True,
                )
                osb = out_pool.tile([P, 512], DT, tag="osbh")
                if ob == 0:
                    nc.scalar.copy(osb[:], pso[:])
                    nc.scalar.dma_start(out[tt * P : (tt + 1) * P, half], osb[:])
                else:
                    nc.vector.tensor_copy(osb[:], pso[:])
                    nc.sync.dma_start(out[tt * P : (tt + 1) * P, half], osb[:])
        for f in outproj_fills(NB - 1, final=True):
            f()


_BUILD_CACHE = {}


def build():
    if "nc" in _BUILD_CACHE:
        return _BUILD_CACHE["nc"]
    nc = bacc.Bacc("TRN2", target_bir_lowering=False, debug=False)
    aps = {
        "xT": nc.dram_tensor("xT", [P, NB * KD * 512], DT, kind="ExternalInput").ap(),
        "wq": nc.dram_tensor("wq", [P, KD * WCOLS], DT, kind="ExternalInput").ap(),
        "wk": nc.dram_tensor("wk", [P, KD * WCOLS], DT, kind="ExternalInput").ap(),
        "wv": nc.dram_tensor("wv", [P, KD * WCOLS], DT, kind="ExternalInput").ap(),
        "wo": nc.dram_tensor("wo", [WCOLS, D], DT, kind="ExternalInput").ap(),
        "tri": nc.dram_tensor("tri", [P, P], DT, kind="ExternalInput").ap(),
        "out": nc.dram_tensor("out", [S, D], DT, kind="ExternalOutput").ap(),
    }
    with tile.TileContext(nc) as tc:
        _emit(tc, aps)
    nc.compile()
    _BUILD_CACHE["nc"] = nc
    return nc


def make_tri() -> np.ndarray:
    """tri[dj, t] = 1 if dj <= t else 0 (causal keep within a 128 block)."""
    dj = np.arange(P)[:, None]
    t = np.arange(P)[None, :]
    return prep(np.where(dj <= t, 1.0, 0.0).astype(np.float32))


def swz_w(w):
    """[D, WCOLS] -> [P, KD*WCOLS]: SBUF-layout swizzle so the DMA is contiguous."""
    return np.ascontiguousarray(
        w.reshape(KD, P, WCOLS).transpose(1, 0, 2).reshape(P, KD * WCOLS)
    )


def swz_x(xb):
    """x[b] [S, D] -> xT swizzled [P, NB*KD*512] matching xc_all layout."""
    xT = xb.T  # [D, S]
    return np.ascontiguousarray(
        xT.reshape(KD, P, NB, 512).transpose(1, 2, 0, 3).reshape(P, NB * KD * 512)
    )


def make_in_maps(x, w_qkv, w_out):
    tri = make_tri()
    scale = 1.0 / np.sqrt(HD)
    in_maps = []
    for c in range(NCORES):
        b, g = c // 4, c % 4
        cs = slice(g * WCOLS, (g + 1) * WCOLS)
        in_maps.append(
            {
                "xT": prep(swz_x(x[b])),
                "wq": prep(swz_w(w_qkv[:, g * WCOLS : (g + 1) * WCOLS] * scale)),
                "wk": prep(swz_w(w_qkv[:, D + g * WCOLS : D + (g + 1) * WCOLS])),
                "wv": prep(swz_w(w_qkv[:, 2 * D + g * WCOLS : 2 * D + (g + 1) * WCOLS])),
                "wo": prep(w_out[cs, :]),
                "tri": tri,
            }
        )
    return in_maps


def kernel(x, w_qkv, w_out, _trace=False):
    nc = build()
    in_maps = make_in_maps(
        np.asarray(x, np.float32), np.asarray(w_qkv, np.float32),
        np.asarray(w_out, np.float32),
    )
    res = bass_utils.run_bass_kernel_spmd(
        nc, in_maps, core_ids=list(range(NCORES)), trace=_trace
    )
    outs = [np.asarray(res.results[c]["out"], np.float32) for c in range(NCORES)]
    full = np.stack(
        [sum(outs[b * 4 : (b + 1) * 4][1:], outs[b * 4]) for b in range(B)], axis=0
    )
    if _trace:
        kernel.last_results = res
    return full.astype(np.float32)

